# revision 24
# baseline (speedup 1.0000x reference)
"""AttentiveManifoldMixer Trainium2 kernel (8-core data parallel over batch).

Math: with W3[c,i,j] = conv_w[c*64+i, j], B = conv_b.reshape(C, C),
  s[b]       = sigmoid(fc2 @ relu(fc1 @ mean_hw(x[b])))
  out[b,c,p] = sum_{i,j} W3[c,i,j] * s[b,j] * x[b,i,p] * x[b,j,p]
               + sum_i B[c,i] * x[b,i,p]

The quadratic form runs over unordered channel pairs grouped by cyclic
offset d = (j-i) mod 64, split across both elementwise engines (measured
HW rates: DVE tensor_tensor ~2x its spec formula due to drain
serialization; matmul ~280ns per N=512 col-paired issue):

- d=0..23 (12 "product" chunks, DVE): lanes hold x_i*x_j, built by
  tensor_tensor from rotated copies of the doubled bf16 x.  Rotations
  are staged through a doubled DRAM image [x;x] and fetched with a few
  large batched DMAs (per-DMA fixed cost ~2us makes many small
  SBUF->SBUF copies the bottleneck otherwise).
- d=24..32 (5 "sum" chunks, PE+ACT): lanes hold (x_i+x_j)^2 via a 0/1
  basis matmul against [x;x] then a scalar-engine Square;
  x_i x_j = ((x_i+x_j)^2 - x_i^2 - x_j^2)/2 folds into the weights and
  the -x^2 corrections ride on 64 lanes (4*x_i^2) of the last sum
  chunk, whose s-dependent weights are an 18-term gather/scale/reduce.

GEMM: every chunk contracts K=128 lanes -> M=64 channels over N=512
pixel blocks; blocks j and j+4 sit on PSUM partition halves of 4 full
banks via column-group tile_position (0,0)/(0,64).  conv_b is one more
K=128 bf16 chunk ([B.T;0] against [x;x]).

The timing program (niter) runs an UNROLL=2 software pipeline: the loop
body holds two complete executions on ping-pong buffers, so one
execution's DMA staging (x load, cast, image write, rot reads) hides
under the other's compute.  For_i(niter/2) preserves "niter executions".
"""
import sys

sys.path.insert(0, "/opt/trn_rl_repo")

import numpy as np
import ml_dtypes

B, C, H, W = 8, 64, 64, 64
P = H * W                  # 4096 pixels per sample
MID = C // 4
NPROD = 12                 # product chunks: (k,l), k=0..3, l=0..2 -> d=0..23
NSUM = 5                   # sum chunks: d = 24+2mc+qhi; mc=4/qhi=1 = x^2 lanes
NCH = NPROD + NSUM
E_OFFS = [0] + list(range(24, 41))   # x^2-correction gather offsets (18)
NE = len(E_OFFS)
NSUB = 512
HW2 = P // 2
N_CORES = 8

_CACHE = {}


def _lane_maps():
    i_idx = np.zeros((NCH, 128), np.int64)
    j_idx = np.zeros((NCH, 128), np.int64)
    mult = np.ones((NCH, 128), np.float64)
    x2lane = np.zeros((NCH, 128), bool)
    for m in range(NPROD):
        k, l = divmod(m, 3)
        for q in range(128):
            qhi, qlo = divmod(q, 64)
            i_idx[m, q] = (qlo - 6 * k) % 64
            j_idx[m, q] = (i_idx[m, q] + 6 * k + 2 * l + qhi) % 64
    for mc in range(NSUM):
        m = NPROD + mc
        for q in range(128):
            qhi, qlo = divmod(q, 64)
            if mc == 4 and qhi == 1:
                i_idx[m, q] = j_idx[m, q] = qlo
                x2lane[m, q] = True
            else:
                d = 24 + 2 * mc + qhi
                i_idx[m, q] = qlo
                j_idx[m, q] = (qlo + d) % 64
                if d == 32:
                    mult[m, q] = 2.0
    return i_idx, j_idx, mult, x2lane


def _host_inputs(conv_w, fc1_w, fc2_w, conv_b):
    """Per-core constant inputs, packed into three DMA-friendly tensors."""
    w3 = conv_w.reshape(C, C, C).astype(np.float64)  # [c, i, j]
    i_idx, j_idx, mult, x2lane = _lane_maps()
    a12 = np.zeros((128, 2, NCH, C))
    CA = np.zeros((C, C, C))  # x^2 corrections [c, i_target, j_sidx]
    for m in range(NCH):
        is_sum = m >= NPROD
        for q in range(128):
            if x2lane[m, q]:
                continue
            i, j = i_idx[m, q], j_idx[m, q]
            if not is_sum:
                a12[q, 0, m, :] = w3[:, i, j]
                a12[q, 1, m, :] = w3[:, j, i] if i != j else 0.0
            else:
                h1 = w3[:, i, j] / (2 * mult[m, q])
                h2 = w3[:, j, i] / (2 * mult[m, q])
                a12[q, 0, m, :] = h1
                a12[q, 1, m, :] = h2
                CA[:, i, j] -= h1
                CA[:, i, i] -= h2
                CA[:, j, j] -= h1
                CA[:, j, i] -= h2
    kall = np.zeros((C, NE, C))  # [i, e, c]; x^2 feature is 4*x_i^2
    for t, e in enumerate(E_OFFS):
        for i in range(C):
            kall[i, t, :] = CA[:, i, (i + e) % 64] / 4.0
    uall = np.zeros((128, NSUM, 128))
    for mc in range(NSUM):
        m = NPROD + mc
        for q in range(128):
            uall[q % 64, mc, q] += 1.0
            uall[64 + j_idx[m, q], mc, q] += 1.0
    # f32 pack: kall | fc1t | fc2t
    kf = np.zeros((C, NE * C + MID + C))
    kf[:, :NE * C] = kall.reshape(C, -1)
    kf[:, NE * C:NE * C + MID] = fc1_w.T / float(P)
    kf[0:MID, NE * C + MID:] = fc2_w.T
    # gather permutations: sgb col t = s[perm_t(q)] via tiny matmuls
    perms = np.zeros((C, 12, 128))
    for t in range(12):
        for q in range(128):
            qhi, qlo = divmod(q, 64)
            if t < 3:
                pi = (qlo + 2 * t + qhi) % 64
            elif t < 7:
                pi = (qlo - 6 * (t - 3)) % 64
            else:
                pi = (qlo + 24 + 2 * (t - 7) + qhi) % 64
            perms[pi, t, q] = 1.0
    # bf16 pack: uall | conv_b-as-[B.T;0] | perms
    ub = np.zeros((128, NSUM * 128 + C + 12 * 128))
    ub[:, :NSUM * 128] = uall.reshape(128, -1)
    ub[0:C, NSUM * 128:NSUM * 128 + C] = conv_b.reshape(C, C).T
    ub[0:C, NSUM * 128 + C:] = perms.reshape(C, -1)
    return {
        "a12": np.ascontiguousarray(a12, ml_dtypes.bfloat16),
        "kf": np.ascontiguousarray(kf, np.float32),
        "ub": np.ascontiguousarray(ub, ml_dtypes.bfloat16),
    }


def _build_program(niter=None, unroll=None):
    import contextlib

    import concourse.bacc as bacc
    import concourse.bass as bass
    from concourse import mybir
    from concourse.tile import TileContext

    nc = bacc.Bacc("TRN2", target_bir_lowering=False, debug=False)
    dt = mybir.dt
    AF = mybir.ActivationFunctionType
    UNROLL = unroll if unroll else (4 if niter else 1)

    x_d = nc.dram_tensor("x", [C, P], dt.float32r, kind="ExternalInput")
    a12_d = nc.dram_tensor("a12", [128, 2, NCH, C], dt.bfloat16,
                           kind="ExternalInput")
    kf_d = nc.dram_tensor("kf", [C, NE * C + MID + C], dt.float32,
                          kind="ExternalInput")
    ub_d = nc.dram_tensor("ub", [128, NSUM * 128 + C + 12 * 128], dt.bfloat16,
                          kind="ExternalInput")
    out_d = nc.dram_tensor("out", [C, P], dt.float32, kind="ExternalOutput")

    hsls = [slice(0, HW2), slice(HW2, P)]

    with TileContext(nc) as tc:
        with tc.tile_pool(name="single", bufs=1) as single, \
             tc.tile_pool(name="dram", bufs=1, space="DRAM") as dpool, \
             tc.tile_pool(name="xfp", bufs=2) as xfp, \
             tc.tile_pool(name="feat", bufs=2) as featp, \
             tc.tile_pool(name="sqp", bufs=2) as sqp, \
             tc.tile_pool(name="outs", bufs=1) as outsp, \
             tc.tile_pool(name="psum", bufs=2, space="PSUM") as psum:

            # ---- constants: loaded once per program, resident in SBUF ----
            a12s = single.tile([128, 2, NCH, C], dt.bfloat16)
            nc.scalar.dma_start(out=a12s, in_=a12_d.ap())
            kfs = single.tile([C, NE * C + MID + C], dt.float32)
            nc.scalar.dma_start(out=kfs, in_=kf_d.ap())
            ubs = single.tile([128, NSUM * 128 + C + 12 * 128], dt.bfloat16)
            nc.scalar.dma_start(out=ubs, in_=ub_d.ap())
            a1s = a12s[:, 0, :, :]
            a2s = a12s[:, 1, :, :]
            kalls = kfs[:, :NE * C].rearrange("p (e c) -> p e c", e=NE)
            f1s = kfs[:, NE * C:NE * C + MID]
            f2s = kfs[0:MID, NE * C + MID:]
            ualls = ubs[:, :NSUM * 128].rearrange("p (m q) -> p m q", m=NSUM)
            ids = ubs[:, NSUM * 128:NSUM * 128 + C]
            perms = ubs[0:C, NSUM * 128 + C:].rearrange(
                "p (t q) -> p t q", t=12)
            # shared fold scratch (WAR deps order the phases)
            t1 = single.tile([128, NCH, C], dt.float32)
            t2 = single.tile([128, NCH, C], dt.float32)
            tw2 = single.tile([C, C, NE], dt.bfloat16)

            # per-phase tile sets (2 buffer sets, reused modulo 2)
            T = []
            for ph in range(min(UNROLL, 2)):
                Sn = lambda n: f"{n}_{ph}"
                t = dict(
                    xb2=single.tile([128, P], dt.bfloat16, name=Sn("xb2")),
                    avcat=single.tile([128, 3, P], dt.bfloat16,
                                      name=Sn("av")),
                    bvcat=single.tile([128, 3, P], dt.bfloat16,
                                      name=Sn("bv")),
                    wc=single.tile([128, NCH, C], dt.bfloat16,
                                   name=Sn("wc")),
                    xb2d=dpool.tile([128, P], dt.bfloat16, name=Sn("xb2d")),
                    s_int=dpool.tile([2 * C], dt.float32, name=Sn("sint")),
                    sums=[single.tile([C, 1], dt.float32,
                                      name=Sn(f"sums{h}")) for h in range(2)],
                    y1=single.tile([MID, 1], dt.float32, name=Sn("y1")),
                    svec=single.tile([C, 1], dt.bfloat16, name=Sn("svec")),
                    sgb=single.tile([128, 12], dt.float32, name=Sn("sgb")),
                    s_w2=single.tile([C, NE - 1], dt.float32,
                                     name=Sn("sw2")),
                    w2red=single.tile([C, C], dt.float32, name=Sn("w2red")),
                    w2b=single.tile([C, C], dt.bfloat16, name=Sn("w2b")),
                )
                T.append(t)

            def preamble_steps(ph):
                """List of emission closures staging + folding phase ph."""
                t = T[ph]
                xb2, xb2d = t["xb2"], t["xb2d"]

                def ld(h):
                    xfh = xfp.tile([C, HW2], dt.float32r, tag="xf",
                                   name="xfh")
                    nc.sync.dma_start(out=xfh, in_=x_d.ap()[:, hsls[h]])
                    nc.scalar.activation(xb2[0:C, hsls[h]], xfh, AF.Copy,
                                         accum_out=t["sums"][h])

                def dup_wr():
                    nc.scalar.dma_start(out=xb2[C:128, :], in_=xb2[0:C, :])
                    nc.sync.dma_start(out=xb2d[0:C, :], in_=xb2[0:C, :])
                    nc.sync.dma_start(out=xb2d[C:128, :], in_=xb2[0:C, :])

                def bread():
                    for hi in range(2):
                        nc.sync.dma_start(
                            out=t["bvcat"][64 * hi:64 * hi + 64, :, :],
                            in_=bass.AP(tensor=xb2d.tensor,
                                        offset=xb2d.offset + hi * P,
                                        ap=[[P, 64], [2 * P, 3], [1, P]]))

                def aread():
                    for hi in range(2):
                        nc.scalar.dma_start(
                            out=t["avcat"][64 * hi:64 * hi + 64, :, :],
                            in_=bass.AP(tensor=xb2d.tensor,
                                        offset=xb2d.offset + 46 * P,
                                        ap=[[P, 64], [6 * P, 3], [1, P]]))

                def se():
                    ps1 = psum.tile([MID, 1], dt.float32, tag="s1",
                                    name="ps1")
                    for h in range(2):
                        nc.tensor.matmul(ps1, f1s, t["sums"][h],
                                         start=(h == 0), stop=(h == 1))
                    nc.scalar.activation(t["y1"], ps1, AF.Relu)
                    ps2 = psum.tile([C, 1], dt.float32, tag="s1", name="ps2")
                    nc.tensor.matmul(ps2, f2s, t["y1"], start=True, stop=True)
                    nc.scalar.activation(t["svec"], ps2, AF.Sigmoid)
                    # gather s windows via 12 tiny permutation matmuls
                    psb = psum.tile([128, 12], dt.float32, tag="s1",
                                    name="psb")
                    for g in range(12):
                        nc.tensor.matmul(psb[:, g:g + 1], perms[:, g, :],
                                         t["svec"], start=True, stop=True)
                    nc.scalar.copy(t["sgb"], psb)
                    # x^2-correction gathers (not latency-critical)
                    nc.gpsimd.dma_start(out=t["s_int"][0:C][:, None],
                                        in_=t["svec"])
                    nc.gpsimd.dma_start(out=t["s_int"][C:2 * C][:, None],
                                        in_=t["svec"])
                    nc.gpsimd.dma_start(
                        out=t["s_w2"],
                        in_=bass.AP(tensor=t["s_int"].tensor,
                                    offset=t["s_int"].offset + E_OFFS[1],
                                    ap=[[1, 64], [1, NE - 1]]))

                def folds1():
                    sgb = t["sgb"]
                    for l in range(3):
                        nc.scalar.mul(t1[:, l:NPROD:3, :],
                                      a1s[:, l:NPROD:3, :], sgb[:, l:l + 1])
                    for mc in range(NSUM):
                        nc.scalar.mul(t1[:, NPROD + mc, :],
                                      a1s[:, NPROD + mc, :],
                                      sgb[:, 7 + mc:8 + mc])

                def folds2():
                    sgb = t["sgb"]
                    for k in range(4):
                        nc.scalar.mul(t2[:, 3 * k:3 * k + 3, :],
                                      a2s[:, 3 * k:3 * k + 3, :],
                                      sgb[:, 3 + k:4 + k])
                    nc.scalar.mul(t2[:, NPROD:NCH, :], a2s[:, NPROD:NCH, :],
                                  sgb[:, 3:4])

                def foldsw2():
                    for g in range(NE):
                        sc = (t["sgb"][0:C, 3:4] if g == 0
                              else t["s_w2"][:, g - 1:g])
                        nc.scalar.mul(tw2[:, :, g], kalls[:, g, :], sc)

                return [lambda: ld(0), lambda: (ld(1), dup_wr()), bread,
                        aread, se, folds1, folds2, foldsw2]

            def emit_main(ph, pre_steps):
                t = T[ph]
                xb2, wc = t["xb2"], t["wc"]
                avcat, bvcat = t["avcat"], t["bvcat"]
                # fold add first in DVE order (reads shared t1/t2)
                nc.vector.tensor_add(
                    wc.rearrange("p a b -> p (a b)"),
                    t1.rearrange("p a b -> p (a b)"),
                    t2.rearrange("p a b -> p (a b)"))

                obank = [psum.tile([128, NSUB], dt.float32, tag="ob", bufs=4,
                                   name=f"ob{jj}") for jj in range(4)]

                def chunk_gemms(m, rhs, start, stop):
                    for jj in range(4):
                        for h in range(2):
                            nc.tensor.matmul(
                                obank[jj][64 * h:64 * h + 64, :],
                                wc[:, m, :],
                                rhs[:, 2048 * h + NSUB * jj:
                                    2048 * h + NSUB * (jj + 1)],
                                start=start, stop=stop,
                                skip_group_check=True,
                                tile_position=(0, 64 * h))

                sumwork = []
                for mc in range(NSUM):
                    sumwork.extend(("s1", mc, cb) for cb in range(4))
                    sumwork.append(("s2", mc))
                sumsq_t = {}

                def do_sumwork():
                    op = sumwork.pop(0)
                    if op[0] == "s1":
                        _, mc, cb = op
                        if cb == 0:
                            sumsq_t[mc] = sqp.tile([128, P], dt.bfloat16,
                                                   tag="sq", name="sq")
                        s1t = psum.tile([128, 1024], dt.float32, tag="s1",
                                        name="s1t")
                        c0 = cb * 1024
                        for n in range(2):
                            nc.tensor.matmul(
                                s1t[:, n * NSUB:(n + 1) * NSUB],
                                ualls[:, mc, :],
                                xb2[:, c0 + n * NSUB:c0 + (n + 1) * NSUB],
                                start=True, stop=True)
                        nc.scalar.activation(sumsq_t[mc][:, c0:c0 + 1024],
                                             s1t, AF.Square)
                    else:
                        mc = op[1]
                        chunk_gemms(NPROD + mc, sumsq_t[mc], start=False,
                                    stop=(mc == NSUM - 1))

                pend = []

                def flush():
                    m0, f0 = pend.pop(0)
                    chunk_gemms(m0, f0, start=(m0 == 0), stop=False)
                    if m0 == 5:
                        for jj in range(4):
                            for h in range(2):
                                col = 2048 * h + NSUB * jj
                                nc.tensor.matmul(
                                    obank[jj][64 * h:64 * h + 64, :], ids,
                                    xb2[:, col:col + NSUB], start=False,
                                    stop=False, skip_group_check=True,
                                    tile_position=(0, 64 * h))
                    for _ in range(2):
                        if sumwork:
                            do_sumwork()
                    if pre_steps:
                        pre_steps.pop(0)()
                    if pre_steps:
                        pre_steps.pop(0)()

                for _ in range(3):
                    do_sumwork()
                if pre_steps:
                    pre_steps.pop(0)()
                for m in range(NPROD):
                    k, l = divmod(m, 3)
                    f = featp.tile([128, P], dt.bfloat16, tag="f", name="f")
                    a_ap = xb2 if k == 0 else avcat[:, 3 - k, :]
                    for hsl in hsls:
                        nc.vector.tensor_mul(f[:, hsl], a_ap[:, hsl],
                                             bvcat[:, l, :][:, hsl])
                    pend.append((m, f))
                    if m == 8:
                        nc.vector.tensor_reduce(t["w2red"], tw2,
                                                axis=mybir.AxisListType.X,
                                                op=mybir.AluOpType.add)
                        nc.scalar.copy(t["w2b"], t["w2red"])
                        nc.scalar.dma_start(out=wc[C:128, NCH - 1, :],
                                            in_=t["w2b"])
                    if m >= 1:
                        flush()
                while pend:
                    flush()
                while sumwork:
                    do_sumwork()
                while pre_steps:
                    pre_steps.pop(0)()

                ot = outsp.tile([128, 4 * NSUB], dt.float32, tag="ot",
                                name="ot")
                for jj in range(4):
                    nc.scalar.copy(ot[:, NSUB * jj:NSUB * (jj + 1)],
                                   obank[jj])
                nc.scalar.dma_start(out=out_d.ap()[:, 0:2048], in_=ot[0:C, :])
                nc.scalar.dma_start(out=out_d.ap()[:, 2048:P],
                                    in_=ot[C:128, :])

            with (tc.For_i(0, niter // UNROLL, 1,
                           hint_engines=(mybir.EngineType.PE,
                                         mybir.EngineType.DVE,
                                         mybir.EngineType.SP,
                                         mybir.EngineType.Activation,
                                         mybir.EngineType.Pool))
                  if niter else contextlib.nullcontext()):
                if UNROLL == 1:
                    for step in preamble_steps(0):
                        step()
                    emit_main(0, [])
                else:
                    # software pipeline: while phase ph computes, the other
                    # phase's staging + s-chain run from interleaved steps
                    for u in range(UNROLL):
                        emit_main(u % 2, preamble_steps((u + 1) % 2))

    nc.compile()
    return nc


def _get_program(niter=None):
    key = ("nc", niter)
    if key not in _CACHE:
        _CACHE[key] = _build_program(niter)
    return _CACHE[key]


def kernel(x, fc1_w, fc2_w, conv_w, conv_b):
    from concourse.bass_utils import run_bass_kernel_spmd

    x = np.asarray(x, np.float32)
    host = _host_inputs(np.asarray(conv_w, np.float32),
                        np.asarray(fc1_w, np.float32),
                        np.asarray(fc2_w, np.float32),
                        np.asarray(conv_b, np.float32))
    nc = _get_program()
    in_maps = []
    for b in range(N_CORES):
        in_maps.append({"x": np.ascontiguousarray(x[b].reshape(C, P)), **host})
    res = run_bass_kernel_spmd(nc, in_maps, core_ids=list(range(N_CORES)))
    out = np.stack([res.results[b]["out"].reshape(C, H, W)
                    for b in range(N_CORES)], axis=0)
    return out.astype(np.float32)
